# revision 18
# baseline (speedup 1.0000x reference)
"""Cross-view attention Trainium2 kernel (v10).

v10 changes over v9 (each validated by microbenchmarks; v9 measured
~530us/iter, v10 ~475-510us/iter, with the Tile scheduling sim showing
94% TensorE occupancy -- i.e. at the bf16 matmul roofline for the
sustained-load PE clock (~2.0 GHz under 8-core load, 272ns per
K=128/M=128/N=512 matmul measured vs 213ns at nominal 2.4 GHz):
  - phase-1 f32->f16 input conversion rides the SWDGE cast-DMA
    (nc.gpsimd.dma_start with dtype cast) instead of ACT/DVE copy ops:
    removes ~4us/window of ScalarE+VectorE work and 6MB of f32 SBUF
    staging; xb (bf16) is one cheap f16->bf16 DVE copy.
  - epilogue evacuates the 4 acc PSUM banks with plain DVE copies
    BEFORE the l-reciprocal/broadcast chain, so the next chunk's PV
    start=True matmuls are not gated on the epilogue scalar chain.
  - PV matmuls emitted ct-outer (consecutive MMs share a PSUM bank,
    ~3% faster than alternating banks every MM).
  - prefetch/output DMAs consolidated to one 1MB descriptor per chunk.
Rejected after measurement: fp8 DoubleRow PV (1.94x matmul throughput
in isolation, but e4m3's 2^-4 max rel err on v with near-one-hot
softmax rows puts absmax rel err at ~2e-2 = the correctness gate);
psS bufs=4 (PSUM is exactly full: 3 S^T + 4 acc + 1 accl banks).

Reference computation (per sample b):
    q = Wq @ x1 + bq            (D=64, N)      x1 = view1[b] as (C, N)
    k = Wk @ x2 + bk            (D, N)
    v = Wv @ x2 + bv            (C, N)
    S = q^T k                   (N, N)
    P = softmax(S, axis=-1)
    out = v @ P^T               (C, N)
    y = gamma * out + x1

Sharding: data-parallel over batch B=8 across the 8 NeuronCores (one
sample per core), no collectives.

Device algorithm (per core):
  - Projections in the layouts attention needs (qT/kT as (64, N) f16
    duplicated across both partition halves, vT as (m, C) bf16); q and
    k projection matmuls run column-packed (M=64 each, concurrent in
    disjoint PE column halves, separate PSUM banks); PSUM evictions
    balanced across engines (q on VectorE, k and vT on ScalarE, which
    is otherwise idle during phase 1) so projections stay matmul-paced.
  - NO K=1 / single-partition matmuls anywhere: they cost ~100+ us
    each on real hw (vs ~214 ns in the cost model) and were the
    dominant cost of the original baseline (21 ms/iter).  The v-bias
    matmuls are gone (sum_m P*(v+bv) = sum_m P*v + bv*l, so gamma*bv
    is a host-precomputed per-partition constant added free in the
    epilogue via scalar_tensor_tensor); the gamma/l partition
    broadcast uses nc.gpsimd.partition_broadcast instead of a ones
    matmul.
  - Attention computed transposed: S^T tiles (m=128 partitions, n=512
    free) via row-packed pairs of K=64 matmuls (tile_position, 2x
    concurrent); exp on ScalarE (no max subtraction: logits bounded
    ~+-50, exp stays in fp32/bf16 range); P^T tiles feed
    out[c,n] += vT^T @ expS^T accumulated over m in PSUM.
  - Softmax denominator: 4 column-tiled M=1 accumulation chains at PE
    column groups 0/32/64/96 of ONE PSUM bank, batched 4 back-to-back
    per group of 4 m-tiles so they genuinely run concurrently; a
    zero-weights full-partition matmul initializes the bank so all
    chains use start=False (correct under both whole-bank and
    per-partition has_written semantics); the epilogue sums the bank's
    128 rows (quarters + explicit zeros) with one f32r ones-column
    matmul.
  - Epilogue per n-chunk: y = acc * (gamma/l) + gamma*bv + view1 with
    view1 windows prefetched at chunk start; software-pipelined behind
    the next chunk's first S^T group.
  - Phase-1 conversions balanced across engines: xr on ScalarE (idle
    during projections), xb/xqr on VectorE, all split in halves so the
    first channel-chunk matmuls start while the rest converts; persist
    tiles are per-window so attention starts as soon as window 0 is
    projected.
"""

import sys

if "/opt/trn_rl_repo" not in sys.path:
    sys.path.insert(0, "/opt/trn_rl_repo")

import numpy as np

B, C, H, W = 8, 512, 64, 64
D = C // 8            # 64
N = H * W             # 4096
CC = C // 128         # 4 chunks of the channel dim
NCORES = 8

_compiled = {}


def _build(n=N, repeat=1, nwin=512, drop=()):
    from contextlib import ExitStack

    import concourse.mybir as mybir
    import concourse.tile as tile
    from concourse import bacc

    dt = mybir.dt
    f32, f32r, bf16 = dt.float32, dt.float32r, dt.bfloat16
    f16 = dt.float16
    AF = mybir.ActivationFunctionType
    ALU = mybir.AluOpType

    nwin = min(nwin, n)
    nch = n // nwin       # output n-chunks
    mt = n // 128         # m tiles (key/value rows per tile)

    nc = bacc.Bacc("TRN2", target_bir_lowering=False, debug=False)
    v1 = nc.dram_tensor("v1", [C, n], f32, kind="ExternalInput").ap()
    v2 = nc.dram_tensor("v2", [C, n], f32, kind="ExternalInput").ap()
    wqT = nc.dram_tensor("wqT", [C, D], f16, kind="ExternalInput").ap()
    wkT = nc.dram_tensor("wkT", [C, D], f16, kind="ExternalInput").ap()
    wvT = nc.dram_tensor("wvT", [C, C], bf16, kind="ExternalInput").ap()
    bqh = nc.dram_tensor("bqh", [D, 1], f32, kind="ExternalInput").ap()  # bq col
    bkc = nc.dram_tensor("bkc", [D, 1], f32, kind="ExternalInput").ap()  # bk col
    gbv = nc.dram_tensor("gbv", [128, CC], f32, kind="ExternalInput").ap()
    gam = nc.dram_tensor("gam", [1, 1], f32, kind="ExternalInput").ap()
    out = nc.dram_tensor("out", [C, n], f32, kind="ExternalOutput").ap()

    v1p = v1.rearrange("(cc p) n -> p cc n", p=128)
    v2p = v2.rearrange("(cc p) n -> p cc n", p=128)
    outp = out.rearrange("(cc p) n -> p cc n", p=128)

    with tile.TileContext(nc) as tc, ExitStack() as top:
        consts = top.enter_context(tc.tile_pool(name="consts", bufs=1))

        # ---- constants ----
        wq_s = consts.tile([128, CC, D], f16, tag="wq")
        wk_s = consts.tile([128, CC, D], f16, tag="wk")
        wv_s = consts.tile([128, CC, C], bf16, tag="wv")
        bqc_s = consts.tile([D, 1], f32, tag="bqc")   # ACT bias column (bq/2)
        bkc_s = consts.tile([D, 1], f32, tag="bkc")
        gbv_s = consts.tile([128, CC], f32, tag="gbv")  # gamma*bv columns
        gam_s = consts.tile([1, 1], f32, tag="gam")
        ones_col = consts.tile([128, 1], bf16, tag="ones_col")  # K=128, M=1 lhsT (l)
        ones_colr = consts.tile([128, 1], f32r, tag="ones_colr")  # f32r variant (l sum)
        zeros_sq = consts.tile([128, 128], bf16, tag="zeros_sq")  # bank-zeroing lhsT

        with ExitStack() as p0:
            wstp = p0.enter_context(tc.tile_pool(name="wst", bufs=1))
            # weights arrive pre-converted (f16/bf16) from the host: no
            # staging tile, no DVE conversions on the startup critical path
            nc.scalar.dma_start(wq_s[:], wqT.rearrange("(cc p) d -> p cc d", p=128))
            nc.scalar.dma_start(wk_s[:], wkT.rearrange("(cc p) d -> p cc d", p=128))
            nc.scalar.dma_start(wv_s[:], wvT.rearrange("(cc p) c -> p cc c", p=128))

            nc.scalar.dma_start(bqc_s[:], bqh[:])
            nc.scalar.dma_start(bkc_s[:], bkc[:])
            nc.scalar.dma_start(gbv_s[:], gbv[:])
            nc.scalar.dma_start(gam_s[:], gam[:])

            ones_f32 = wstp.tile([128, 128], f32, tag="ones_f32")
            nc.vector.memset(ones_f32[:], 1.0)
            nc.vector.tensor_copy(ones_col[:], ones_f32[:, :1])
            nc.vector.tensor_copy(ones_colr[:], ones_f32[:, :1])
            nc.vector.memset(zeros_sq[:], 0.0)

        def emit_rep(rep):
            with ExitStack() as rctx:
                per = rctx.enter_context(tc.tile_pool(name=f"persist{rep}", bufs=1))
                # per-window persist tiles (instead of one monolithic tile
                # per tensor): phase-2 reads then depend only on the window
                # that produced them, so attention on window 0 can start
                # while later windows are still being projected
                qT_t = [per.tile([128, nwin], f16, tag=f"qT{j}", name=f"qT{j}")
                        for j in range(nch)]
                kT_t = [per.tile([128, nwin], f16, tag=f"kT{j}", name=f"kT{j}")
                        for j in range(nch)]
                vT_t = [per.tile([128, nwin // 128, C], bf16, tag=f"vT{j}",
                                 name=f"vT{j}") for j in range(nch)]

                # ================= phase 1: projections =================
                if "proj" in drop:
                    for j in range(nch):
                        nc.vector.memset(qT_t[j][:], 0.01)
                        nc.vector.memset(kT_t[j][:], 0.01)
                        nc.vector.memset(vT_t[j][:], 0.01)
                with ExitStack() as p1:
                    nch1 = 0 if "proj" in drop else nch
                    xrp = p1.enter_context(tc.tile_pool(name=f"xrp{rep}", bufs=3))
                    ps1 = p1.enter_context(
                        tc.tile_pool(name=f"ps1{rep}", bufs=2, space="PSUM")
                    )

                    # view2 windows -> kT (f16) + vT (bf16), one stream.
                    # f32->f16 cast rides the SWDGE DMA (gpsimd queue), so no
                    # on-chip conversion ops; xb (bf16 for the v matmuls) is
                    # one cheap f16->bf16 DVE copy.
                    for j in range(nch1):
                        jw = slice(j * nwin, (j + 1) * nwin)
                        xr = xrp.tile([128, CC, nwin], f16, tag="xr")
                        nc.gpsimd.dma_start(xr[:, :2, :], v2p[:, :2, jw])
                        nc.gpsimd.dma_start(xr[:, 2:, :], v2p[:, 2:, jw])
                        xb = xrp.tile([128, CC, nwin], bf16, tag="xb")
                        nc.vector.tensor_copy(xb[:, :2, :], xr[:, :2, :])
                        nc.vector.tensor_copy(xb[:, 2:, :], xr[:, 2:, :])

                        # view1 window -> q stream (overlapped with v2 stream)
                        xqr = xrp.tile([128, CC, nwin], f16, tag="xqr")
                        nc.gpsimd.dma_start(xqr[:, :2, :], v1p[:, :2, jw])
                        nc.gpsimd.dma_start(xqr[:, 2:, :], v1p[:, 2:, jw])

                        # q and k projections column-packed into disjoint PE
                        # column halves (concurrent), each chain on its OWN
                        # PSUM bank so both can use start=True safely
                        psq = ps1.tile([128, nwin], f32, tag="psq")
                        psk = ps1.tile([128, nwin], f32, tag="psk")
                        for cc in range(CC):
                            nc.tensor.matmul(
                                psq[0:64, :],
                                wq_s[:, cc, :],
                                xqr[:, cc, :],
                                start=(cc == 0),
                                stop=(cc == CC - 1),
                                tile_position=(0, 0),
                            )
                            nc.tensor.matmul(
                                psk[64:128, :],
                                wk_s[:, cc, :],
                                xr[:, cc, :],
                                start=(cc == 0),
                                stop=(cc == CC - 1),
                                tile_position=(0, 64),
                            )
                        nc.vector.tensor_scalar_add(
                            qT_t[j][:64, :], psq[0:64, :], bqc_s[:]
                        )
                        nc.sync.dma_start(qT_t[j][64:128, :], qT_t[j][:64, :])
                        nc.scalar.activation(
                            kT_t[j][:64, :], psk[64:128, :], AF.Identity,
                            bias=bkc_s[:],
                        )
                        nc.sync.dma_start(kT_t[j][64:128, :], kT_t[j][:64, :])

                        # v projection (no bias matmul: bv folded into the
                        # epilogue as gamma*bv)
                        for mi in range(nwin // 128):
                            miw = slice(mi * 128, (mi + 1) * 128)
                            psv = ps1.tile([128, C], f32, tag="psv")
                            for cc in range(CC):
                                nc.tensor.matmul(
                                    psv[:],
                                    xb[:, cc, miw],
                                    wv_s[:, cc, :],
                                    start=(cc == 0),
                                    stop=(cc == CC - 1),
                                )
                            nc.scalar.activation(vT_t[j][:, mi, :], psv[:], AF.Copy)

                # ================= phase 2: attention =================
                with ExitStack() as p2:
                    psS = p2.enter_context(
                        tc.tile_pool(name=f"psS{rep}", bufs=3, space="PSUM")
                    )
                    psA = p2.enter_context(
                        tc.tile_pool(name=f"psA{rep}", bufs=1, space="PSUM")
                    )
                    psL = p2.enter_context(
                        tc.tile_pool(name=f"psL{rep}", bufs=1, space="PSUM")
                    )
                    expp = p2.enter_context(tc.tile_pool(name=f"expp{rep}", bufs=10))
                    aep = p2.enter_context(tc.tile_pool(name=f"aep{rep}", bufs=2))
                    smalls = p2.enter_context(tc.tile_pool(name=f"smalls{rep}", bufs=2))
                    rbp = p2.enter_context(tc.tile_pool(name=f"rbp{rep}", bufs=2))
                    resp = p2.enter_context(tc.tile_pool(name=f"resp{rep}", bufs=2))
                    outp_sb = p2.enter_context(tc.tile_pool(name=f"outp{rep}", bufs=3))

                    def prefetch_v1(j):
                        jw = slice(j * nwin, (j + 1) * nwin)
                        v1c = resp.tile([128, CC, nwin], f32, tag="v1c",
                                        name="v1c")
                        nc.scalar.dma_start(v1c[:], v1p[:, :, jw])
                        return v1c

                    def emit_epilogue(j, accs, accl, v1c):
                        if "epi" in drop:
                            return
                        # y = acc * (gamma/l) + gamma*bv + view1
                        jw = slice(j * nwin, (j + 1) * nwin)
                        # evacuate the 4 acc PSUM banks FIRST with plain DVE
                        # copies: the next chunk's PV matmuls reuse these
                        # banks, so their release must not wait for the
                        # l-reciprocal/broadcast chain below
                        a_sbs = []
                        for ct in range(CC):
                            a_sb = aep.tile([128, nwin], f32, tag=f"a{ct}",
                                            name=f"a{ct}_sb")
                            nc.vector.tensor_copy(a_sb[:], accs[ct][:])
                            a_sbs.append(a_sb)
                        # l = sum of the column-tiled partial rows.  The
                        # accl bank is exact zeros outside the 4 quarter
                        # rows (zero-init matmul), so summing all 128 rows
                        # with a ones-column matmul gives l.  (A DVE op
                        # cannot read two PSUM inputs, so no PSUM adds.)
                        acl_sb = rbp.tile([128, nwin], f32r, tag="acl", name="acl_sb")
                        nc.vector.tensor_copy(acl_sb[:], accl[:])
                        l_ps = psL.tile([1, nwin], f32, tag="accl", name="l_ps")
                        nc.tensor.matmul(l_ps[:], ones_colr[:], acl_sb[:], start=True, stop=True)
                        l_sb = smalls.tile([1, nwin], f32, tag="l", name="l_sb")
                        nc.vector.tensor_copy(l_sb[:], l_ps[:])
                        r_sb = smalls.tile([1, nwin], f32, tag="r", name="r_sb")
                        nc.vector.reciprocal(r_sb[:], l_sb[:])
                        rg_sb = smalls.tile([1, nwin], f32, tag="rg", name="rg_sb")
                        nc.vector.tensor_scalar_mul(rg_sb[:], r_sb[:], gam_s[:])
                        # broadcast gamma/l across partitions on the idle
                        # GPSIMD engine (avoids a K=1 matmul + PSUM round
                        # trip; K=1 matmuls are pathologically slow on hw)
                        rb_sb = rbp.tile([128, nwin], f32, tag="rb", name="rb_sb")
                        nc.gpsimd.partition_broadcast(rb_sb[:], rg_sb[:])
                        o_sb = outp_sb.tile([128, CC, nwin], f32, tag="o",
                                            name="o_sb")
                        for ct in range(CC):
                            t_sb = outp_sb.tile([128, nwin], f32, tag="t", name="t_sb")
                            nc.vector.tensor_mul(t_sb[:], a_sbs[ct][:], rb_sb[:])
                            # o = (t + gamma*bv[ct]) + v1
                            nc.vector.scalar_tensor_tensor(
                                o_sb[:, ct, :], t_sb[:], gbv_s[:, ct : ct + 1],
                                v1c[:, ct, :],
                                ALU.add, ALU.add,
                            )
                        # one 1MB output DMA per chunk (descriptor-efficient),
                        # alternating queues across chunks
                        [nc.sync, nc.scalar, nc.gpsimd][j % 3].dma_start(
                            outp[:, :, jw], o_sb[:]
                        )

                    ex_zero_rhs = vT_t[0][:, 0, :]
                    pend_epi = None
                    ngrp = mt // 4
                    exc = None
                    if "st" in drop:
                        exc = expp.tile([128, nwin], bf16, tag="exc", name="exc")
                        nc.vector.memset(exc[:], 0.01)
                    for j in range(nch):
                        jw = slice(j * nwin, (j + 1) * nwin)
                        # one PSUM tile (= one full bank) per output c-chunk:
                        # accumulation groups must not share a bank (start=True
                        # clears the whole bank's has_written bits)
                        accs = [
                            psA.tile([128, nwin], f32, tag=f"acc{ct}", name=f"acc{ct}")
                            for ct in range(CC)
                        ]
                        accl = psL.tile([128, nwin], f32, tag="accl")
                        v1cs = prefetch_v1(j)
                        # software pipeline over GROUPS of 4 m-tiles: issue
                        # S^T (row-packed pairs) + exp of group g before the
                        # P.V matmuls of group g-1, so ScalarE's exp overlaps
                        # TensorE's P.V.  The 4 denominator matmuls of a
                        # group are emitted back-to-back at PE column groups
                        # 0/32/64/96 (disjoint cells -> run concurrently).
                        prev_exs = None

                        def emit_st_pair(g, pi):
                            # one row-packed pair of S^T matmuls + their exps
                            out = []
                            if "st" in drop:
                                return [exc, exc]
                            for half in (0, 1):
                                m = 4 * g + 2 * pi + half
                                mj, ml = divmod(m, nwin // 128)
                                mw = slice(ml * 128, (ml + 1) * 128)
                                hp = slice(64 * half, 64 * half + 64)
                                st = psS.tile([128, nwin], f32, tag="st", name="st")
                                nc.tensor.matmul(
                                    st[:],
                                    kT_t[mj][hp, mw],
                                    qT_t[j][hp, :],
                                    start=True,
                                    stop=True,
                                    tile_position=(64 * half, 0),
                                )
                                ex = expp.tile([128, nwin], bf16, tag="ex", name="ex")
                                nc.scalar.activation(ex[:], st[:], AF.Exp)
                                out.append(ex)
                            return out

                        def emit_pv(g, lo, hi, exs):
                            # ct-outer: consecutive MMs share a PSUM bank
                            # (measured ~3% faster than alternating banks)
                            for ct in range(CC):
                                for i in range(lo, hi):
                                    m = 4 * g + i
                                    if "pv" in drop and m != 0:
                                        continue
                                    mj, ml = divmod(m, nwin // 128)
                                    nc.tensor.matmul(
                                        accs[ct][:],
                                        vT_t[mj][:, ml, ct * 128 : (ct + 1) * 128],
                                        exs[i][:],
                                        start=(m == 0),
                                        stop=(m == mt - 1 or "pv" in drop),
                                    )

                        for g in range(ngrp + 1):
                            exs = []
                            # interleave emission: S^T pair A, then half of
                            # the previous group's P.V, then pair B, then the
                            # rest + the denominator batch.  Keeps the 3rd/4th
                            # S^T (whose PSUM banks may still be feeding exp)
                            # from head-of-line-blocking 16 ready P.V matmuls
                            # in the PE FIFO.
                            if g < ngrp:
                                exs += emit_st_pair(g, 0)
                            if g == 1:
                                if pend_epi is not None:
                                    emit_epilogue(*pend_epi)
                                    pend_epi = None
                                # zero-weights matmul writes explicit zeros
                                # to the whole accl bank (start=True), so
                                # the column-tiled denominator chains can
                                # all accumulate with start=False (correct
                                # under both whole-bank and per-partition
                                # has_written semantics).  Emitted after the
                                # pipelined epilogue of the previous chunk
                                # so the shared psL slot is read first.
                                nc.tensor.matmul(
                                    accl[:], zeros_sq[:], ex_zero_rhs[:, :nwin],
                                    start=True, stop=("accl" in drop),
                                    skip_group_check=True,
                                )
                            if g > 0:
                                emit_pv(g - 1, 0, 2, prev_exs)
                            if g < ngrp:
                                exs += emit_st_pair(g, 1)
                            if g > 0:
                                emit_pv(g - 1, 2, 4, prev_exs)
                                if "accl" not in drop:
                                    for i in range(4):
                                        m = 4 * (g - 1) + i
                                        nc.tensor.matmul(
                                            accl[32 * i : 32 * i + 1, :],
                                            ones_col[:],
                                            prev_exs[i][:],
                                            start=False,
                                            stop=(g == ngrp),
                                            tile_position=(0, 32 * i),
                                            skip_group_check=True,
                                        )
                            prev_exs = exs
                        pend_epi = (j, accs, accl, v1cs)
                    emit_epilogue(*pend_epi)

        if repeat == 1:
            emit_rep(0)
        else:
            with tc.For_i(0, repeat, 1):
                emit_rep(0)

    nc.compile()
    return nc


def _get_nc(n=N, repeat=1):
    key = (n, repeat)
    if key not in _compiled:
        _compiled[key] = _build(n=n, repeat=repeat)
    return _compiled[key]


def _run(nc, view1, view2, Wq, bq, Wk, bk, Wv, bv, gamma, n=N, **spmd_kwargs):
    from concourse.bass_utils import run_bass_kernel_spmd

    b = view1.shape[0]
    f = np.ascontiguousarray
    gamma = np.asarray(gamma).astype(np.float32).reshape(-1)
    gbv = (gamma[0] * np.asarray(bv).astype(np.float32)).reshape(CC, 128).T
    import ml_dtypes

    com = {
        "wqT": f(Wq.T.astype(np.float16)),
        "wkT": f(Wk.T.astype(np.float16)),
        "wvT": f(Wv.T.astype(ml_dtypes.bfloat16)),
        "bqh": f(bq.astype(np.float32).reshape(D, 1)),
        "bkc": f(bk.astype(np.float32).reshape(D, 1)),
        "gbv": f(gbv),
        "gam": f(gamma.reshape(1, 1)),
    }
    in_maps = []
    for i in range(NCORES):
        bi = min(i, b - 1)  # replicate last sample if b < NCORES
        in_maps.append(
            {
                "v1": f(view1[bi].reshape(C, n).astype(np.float32)),
                "v2": f(view2[bi].reshape(C, n).astype(np.float32)),
                **com,
            }
        )
    res = run_bass_kernel_spmd(nc, in_maps, list(range(NCORES)), **spmd_kwargs)
    outs = [res.results[i]["out"] for i in range(b)]
    return np.stack(outs, axis=0)


def kernel(view1, view2, Wq, bq, Wk, bk, Wv, bv, gamma):
    view1 = np.asarray(view1)
    b, c, h, w = view1.shape
    n = h * w
    nc = _get_nc(n=n, repeat=1)
    out = _run(
        nc,
        np.asarray(view1),
        np.asarray(view2),
        np.asarray(Wq),
        np.asarray(bq),
        np.asarray(Wk),
        np.asarray(bk),
        np.asarray(Wv),
        np.asarray(bv),
        np.asarray(gamma),
        n=n,
    )
    return out.reshape(b, c, h, w).astype(np.float32)



# revision 20
# speedup vs baseline: 1.0273x; 1.0273x over previous
"""Cross-view attention Trainium2 kernel (v10).

v10 changes over v9 (each validated by microbenchmarks; v9 measured
~530us/iter, v10 ~475-510us/iter, with the Tile scheduling sim showing
94% TensorE occupancy -- i.e. at the bf16 matmul roofline for the
sustained-load PE clock (~2.0 GHz under 8-core load, 272ns per
K=128/M=128/N=512 matmul measured vs 213ns at nominal 2.4 GHz):
  - phase-1 f32->f16 input conversion rides the SWDGE cast-DMA
    (nc.gpsimd.dma_start with dtype cast) instead of ACT/DVE copy ops:
    removes ~4us/window of ScalarE+VectorE work and 6MB of f32 SBUF
    staging; xb (bf16) is one cheap f16->bf16 DVE copy.
  - epilogue evacuates the 4 acc PSUM banks with plain DVE copies
    BEFORE the l-reciprocal/broadcast chain, so the next chunk's PV
    start=True matmuls are not gated on the epilogue scalar chain.
  - PV matmuls emitted ct-outer (consecutive MMs share a PSUM bank,
    ~3% faster than alternating banks every MM).
  - prefetch/output DMAs consolidated to one 1MB descriptor per chunk.
Rejected after measurement: fp8 DoubleRow PV (1.94x matmul throughput
in isolation, but e4m3's 2^-4 max rel err on v with near-one-hot
softmax rows puts absmax rel err at ~2e-2 = the correctness gate);
psS bufs=4 (PSUM is exactly full: 3 S^T + 4 acc + 1 accl banks).

Reference computation (per sample b):
    q = Wq @ x1 + bq            (D=64, N)      x1 = view1[b] as (C, N)
    k = Wk @ x2 + bk            (D, N)
    v = Wv @ x2 + bv            (C, N)
    S = q^T k                   (N, N)
    P = softmax(S, axis=-1)
    out = v @ P^T               (C, N)
    y = gamma * out + x1

Sharding: data-parallel over batch B=8 across the 8 NeuronCores (one
sample per core), no collectives.

Device algorithm (per core):
  - Projections in the layouts attention needs (qT/kT as (64, N) f16
    duplicated across both partition halves, vT as (m, C) bf16); q and
    k projection matmuls run column-packed (M=64 each, concurrent in
    disjoint PE column halves, separate PSUM banks); PSUM evictions
    balanced across engines (q on VectorE, k and vT on ScalarE, which
    is otherwise idle during phase 1) so projections stay matmul-paced.
  - NO K=1 / single-partition matmuls anywhere: they cost ~100+ us
    each on real hw (vs ~214 ns in the cost model) and were the
    dominant cost of the original baseline (21 ms/iter).  The v-bias
    matmuls are gone (sum_m P*(v+bv) = sum_m P*v + bv*l, so gamma*bv
    is a host-precomputed per-partition constant added free in the
    epilogue via scalar_tensor_tensor); the gamma/l partition
    broadcast uses nc.gpsimd.partition_broadcast instead of a ones
    matmul.
  - Attention computed transposed: S^T tiles (m=128 partitions, n=512
    free) via row-packed pairs of K=64 matmuls (tile_position, 2x
    concurrent); exp on ScalarE (no max subtraction: logits bounded
    ~+-50, exp stays in fp32/bf16 range); P^T tiles feed
    out[c,n] += vT^T @ expS^T accumulated over m in PSUM.
  - Softmax denominator: 4 column-tiled M=1 accumulation chains at PE
    column groups 0/32/64/96 of ONE PSUM bank, batched 4 back-to-back
    per group of 4 m-tiles so they genuinely run concurrently; a
    zero-weights full-partition matmul initializes the bank so all
    chains use start=False (correct under both whole-bank and
    per-partition has_written semantics); the epilogue sums the bank's
    128 rows (quarters + explicit zeros) with one f32r ones-column
    matmul.
  - Epilogue per n-chunk: y = acc * (gamma/l) + gamma*bv + view1 with
    view1 windows prefetched at chunk start; software-pipelined behind
    the next chunk's first S^T group.
  - Phase-1 conversions balanced across engines: xr on ScalarE (idle
    during projections), xb/xqr on VectorE, all split in halves so the
    first channel-chunk matmuls start while the rest converts; persist
    tiles are per-window so attention starts as soon as window 0 is
    projected.
"""

import sys

if "/opt/trn_rl_repo" not in sys.path:
    sys.path.insert(0, "/opt/trn_rl_repo")

import numpy as np

B, C, H, W = 8, 512, 64, 64
D = C // 8            # 64
N = H * W             # 4096
CC = C // 128         # 4 chunks of the channel dim
NCORES = 8

_compiled = {}


def _build(n=N, repeat=1, nwin=512, drop=()):
    from contextlib import ExitStack

    import concourse.mybir as mybir
    import concourse.tile as tile
    from concourse import bacc

    dt = mybir.dt
    f32, f32r, bf16 = dt.float32, dt.float32r, dt.bfloat16
    f16 = dt.float16
    AF = mybir.ActivationFunctionType
    ALU = mybir.AluOpType

    nwin = min(nwin, n)
    nch = n // nwin       # output n-chunks
    mt = n // 128         # m tiles (key/value rows per tile)

    nc = bacc.Bacc("TRN2", target_bir_lowering=False, debug=False)
    v1 = nc.dram_tensor("v1", [C, n], f32, kind="ExternalInput").ap()
    v2 = nc.dram_tensor("v2", [C, n], f32, kind="ExternalInput").ap()
    wqT = nc.dram_tensor("wqT", [C, D], f16, kind="ExternalInput").ap()
    wkT = nc.dram_tensor("wkT", [C, D], f16, kind="ExternalInput").ap()
    wvT = nc.dram_tensor("wvT", [C, C], bf16, kind="ExternalInput").ap()
    bqh = nc.dram_tensor("bqh", [D, 1], f32, kind="ExternalInput").ap()  # bq col
    bkc = nc.dram_tensor("bkc", [D, 1], f32, kind="ExternalInput").ap()  # bk col
    gbv = nc.dram_tensor("gbv", [128, CC], f32, kind="ExternalInput").ap()
    gam = nc.dram_tensor("gam", [1, 1], f32, kind="ExternalInput").ap()
    out = nc.dram_tensor("out", [C, n], f32, kind="ExternalOutput").ap()

    v1p = v1.rearrange("(cc p) n -> p cc n", p=128)
    v2p = v2.rearrange("(cc p) n -> p cc n", p=128)
    outp = out.rearrange("(cc p) n -> p cc n", p=128)

    with tile.TileContext(nc) as tc, ExitStack() as top:
        consts = top.enter_context(tc.tile_pool(name="consts", bufs=1))

        # ---- constants ----
        wq_s = consts.tile([128, CC, D], f16, tag="wq")
        wk_s = consts.tile([128, CC, D], f16, tag="wk")
        wv_s = consts.tile([128, CC, C], bf16, tag="wv")
        bqc_s = consts.tile([D, 1], f32, tag="bqc")   # ACT bias column (bq/2)
        bkc_s = consts.tile([D, 1], f32, tag="bkc")
        gbv_s = consts.tile([128, CC], f32, tag="gbv")  # gamma*bv columns
        gam_s = consts.tile([1, 1], f32, tag="gam")
        ones_col = consts.tile([128, 1], bf16, tag="ones_col")  # K=128, M=1 lhsT (l)
        ones_colr = consts.tile([128, 1], f32r, tag="ones_colr")  # f32r variant (l sum)
        zeros_sq = consts.tile([128, 128], bf16, tag="zeros_sq")  # bank-zeroing lhsT

        with ExitStack() as p0:
            wstp = p0.enter_context(tc.tile_pool(name="wst", bufs=1))
            # weights arrive pre-converted (f16/bf16) from the host: no
            # staging tile, no DVE conversions on the startup critical path
            nc.scalar.dma_start(wq_s[:], wqT.rearrange("(cc p) d -> p cc d", p=128))
            nc.scalar.dma_start(wk_s[:], wkT.rearrange("(cc p) d -> p cc d", p=128))
            nc.scalar.dma_start(wv_s[:], wvT.rearrange("(cc p) c -> p cc c", p=128))

            nc.scalar.dma_start(bqc_s[:], bqh[:])
            nc.scalar.dma_start(bkc_s[:], bkc[:])
            nc.scalar.dma_start(gbv_s[:], gbv[:])
            nc.scalar.dma_start(gam_s[:], gam[:])

            ones_f32 = wstp.tile([128, 128], f32, tag="ones_f32")
            nc.vector.memset(ones_f32[:], 1.0)
            nc.vector.tensor_copy(ones_col[:], ones_f32[:, :1])
            nc.vector.tensor_copy(ones_colr[:], ones_f32[:, :1])
            nc.vector.memset(zeros_sq[:], 0.0)

        def emit_rep(rep):
            with ExitStack() as rctx:
                per = rctx.enter_context(tc.tile_pool(name=f"persist{rep}", bufs=1))
                # per-window persist tiles (instead of one monolithic tile
                # per tensor): phase-2 reads then depend only on the window
                # that produced them, so attention on window 0 can start
                # while later windows are still being projected
                qT_t = [per.tile([128, nwin], f16, tag=f"qT{j}", name=f"qT{j}")
                        for j in range(nch)]
                kT_t = [per.tile([128, nwin], f16, tag=f"kT{j}", name=f"kT{j}")
                        for j in range(nch)]
                vT_t = [per.tile([128, nwin // 128, C], bf16, tag=f"vT{j}",
                                 name=f"vT{j}") for j in range(nch)]

                # ================= phase 1: projections =================
                if "proj" in drop:
                    for j in range(nch):
                        nc.vector.memset(qT_t[j][:], 0.01)
                        nc.vector.memset(kT_t[j][:], 0.01)
                        nc.vector.memset(vT_t[j][:], 0.01)
                with ExitStack() as p1:
                    nch1 = 0 if "proj" in drop else nch
                    xst = p1.enter_context(tc.tile_pool(name=f"xst{rep}", bufs=3))
                    xrp = p1.enter_context(tc.tile_pool(name=f"xrp{rep}", bufs=3))
                    ps1 = p1.enter_context(
                        tc.tile_pool(name=f"ps1{rep}", bufs=2, space="PSUM")
                    )

                    # view2 windows -> kT (f16) + vT (bf16), one stream.
                    # f32->f16 cast rides the SWDGE DMA (gpsimd queue), so no
                    # on-chip conversion ops; xb (bf16 for the v matmuls) is
                    # one cheap f16->bf16 DVE copy.
                    for j in range(nch1):
                        jw = slice(j * nwin, (j + 1) * nwin)
                        xr = xrp.tile([128, CC, nwin], f16, tag="xr")
                        nc.gpsimd.dma_start(xr[:, :2, :], v2p[:, :2, jw])
                        nc.gpsimd.dma_start(xr[:, 2:, :], v2p[:, 2:, jw])
                        xb = xrp.tile([128, CC, nwin], bf16, tag="xb")
                        nc.vector.tensor_copy(xb[:, :2, :], xr[:, :2, :])
                        nc.vector.tensor_copy(xb[:, 2:, :], xr[:, 2:, :])

                        # view1 window -> q stream: f32 on the sync (HWDGE)
                        # queue + DVE convert, so the 16MB of phase-1 input
                        # is split across two DMA queues (the SWDGE cast
                        # queue alone measures ~220GB/s = 72us for 16MB,
                        # which would pace phase 1)
                        xq = xst.tile([128, CC, nwin], f32, tag="xq")
                        nc.sync.dma_start(xq[:, :2, :], v1p[:, :2, jw])
                        nc.sync.dma_start(xq[:, 2:, :], v1p[:, 2:, jw])
                        xqr = xrp.tile([128, CC, nwin], f16, tag="xqr")
                        nc.vector.tensor_copy(xqr[:, :2, :], xq[:, :2, :])
                        nc.vector.tensor_copy(xqr[:, 2:, :], xq[:, 2:, :])

                        # q and k projections column-packed into disjoint PE
                        # column halves (concurrent), each chain on its OWN
                        # PSUM bank so both can use start=True safely
                        psq = ps1.tile([128, nwin], f32, tag="psq")
                        psk = ps1.tile([128, nwin], f32, tag="psk")
                        for cc in range(CC):
                            nc.tensor.matmul(
                                psq[0:64, :],
                                wq_s[:, cc, :],
                                xqr[:, cc, :],
                                start=(cc == 0),
                                stop=(cc == CC - 1),
                                tile_position=(0, 0),
                            )
                            nc.tensor.matmul(
                                psk[64:128, :],
                                wk_s[:, cc, :],
                                xr[:, cc, :],
                                start=(cc == 0),
                                stop=(cc == CC - 1),
                                tile_position=(0, 64),
                            )
                        nc.vector.tensor_scalar_add(
                            qT_t[j][:64, :], psq[0:64, :], bqc_s[:]
                        )
                        nc.sync.dma_start(qT_t[j][64:128, :], qT_t[j][:64, :])
                        nc.scalar.activation(
                            kT_t[j][:64, :], psk[64:128, :], AF.Identity,
                            bias=bkc_s[:],
                        )
                        nc.sync.dma_start(kT_t[j][64:128, :], kT_t[j][:64, :])

                        # v projection (no bias matmul: bv folded into the
                        # epilogue as gamma*bv)
                        for mi in range(nwin // 128):
                            miw = slice(mi * 128, (mi + 1) * 128)
                            psv = ps1.tile([128, C], f32, tag="psv")
                            for cc in range(CC):
                                nc.tensor.matmul(
                                    psv[:],
                                    xb[:, cc, miw],
                                    wv_s[:, cc, :],
                                    start=(cc == 0),
                                    stop=(cc == CC - 1),
                                )
                            nc.scalar.activation(vT_t[j][:, mi, :], psv[:], AF.Copy)

                # ================= phase 2: attention =================
                with ExitStack() as p2:
                    psS = p2.enter_context(
                        tc.tile_pool(name=f"psS{rep}", bufs=3, space="PSUM")
                    )
                    psA = p2.enter_context(
                        tc.tile_pool(name=f"psA{rep}", bufs=1, space="PSUM")
                    )
                    psL = p2.enter_context(
                        tc.tile_pool(name=f"psL{rep}", bufs=1, space="PSUM")
                    )
                    expp = p2.enter_context(tc.tile_pool(name=f"expp{rep}", bufs=10))
                    aep = p2.enter_context(tc.tile_pool(name=f"aep{rep}", bufs=2))
                    smalls = p2.enter_context(tc.tile_pool(name=f"smalls{rep}", bufs=2))
                    rbp = p2.enter_context(tc.tile_pool(name=f"rbp{rep}", bufs=2))
                    resp = p2.enter_context(tc.tile_pool(name=f"resp{rep}", bufs=2))
                    outp_sb = p2.enter_context(tc.tile_pool(name=f"outp{rep}", bufs=3))

                    def prefetch_v1(j):
                        jw = slice(j * nwin, (j + 1) * nwin)
                        v1c = resp.tile([128, CC, nwin], f32, tag="v1c",
                                        name="v1c")
                        nc.scalar.dma_start(v1c[:], v1p[:, :, jw])
                        return v1c

                    def emit_epilogue(j, accs, accl, v1c):
                        if "epi" in drop:
                            return
                        # y = acc * (gamma/l) + gamma*bv + view1
                        jw = slice(j * nwin, (j + 1) * nwin)
                        # evacuate the 4 acc PSUM banks FIRST with plain DVE
                        # copies: the next chunk's PV matmuls reuse these
                        # banks, so their release must not wait for the
                        # l-reciprocal/broadcast chain below
                        a_sbs = []
                        for ct in range(CC):
                            a_sb = aep.tile([128, nwin], f32, tag=f"a{ct}",
                                            name=f"a{ct}_sb")
                            nc.vector.tensor_copy(a_sb[:], accs[ct][:])
                            a_sbs.append(a_sb)
                        # l = sum of the column-tiled partial rows.  The
                        # accl bank is exact zeros outside the 4 quarter
                        # rows (zero-init matmul), so summing all 128 rows
                        # with a ones-column matmul gives l.  (A DVE op
                        # cannot read two PSUM inputs, so no PSUM adds.)
                        acl_sb = rbp.tile([128, nwin], f32r, tag="acl", name="acl_sb")
                        nc.vector.tensor_copy(acl_sb[:], accl[:])
                        l_ps = psL.tile([1, nwin], f32, tag="accl", name="l_ps")
                        nc.tensor.matmul(l_ps[:], ones_colr[:], acl_sb[:], start=True, stop=True)
                        l_sb = smalls.tile([1, nwin], f32, tag="l", name="l_sb")
                        nc.vector.tensor_copy(l_sb[:], l_ps[:])
                        r_sb = smalls.tile([1, nwin], f32, tag="r", name="r_sb")
                        nc.vector.reciprocal(r_sb[:], l_sb[:])
                        rg_sb = smalls.tile([1, nwin], f32, tag="rg", name="rg_sb")
                        nc.vector.tensor_scalar_mul(rg_sb[:], r_sb[:], gam_s[:])
                        # broadcast gamma/l across partitions on the idle
                        # GPSIMD engine (avoids a K=1 matmul + PSUM round
                        # trip; K=1 matmuls are pathologically slow on hw)
                        rb_sb = rbp.tile([128, nwin], f32, tag="rb", name="rb_sb")
                        nc.gpsimd.partition_broadcast(rb_sb[:], rg_sb[:])
                        o_sb = outp_sb.tile([128, CC, nwin], f32, tag="o",
                                            name="o_sb")
                        for ct in range(CC):
                            t_sb = outp_sb.tile([128, nwin], f32, tag="t", name="t_sb")
                            nc.vector.tensor_mul(t_sb[:], a_sbs[ct][:], rb_sb[:])
                            # o = (t + gamma*bv[ct]) + v1
                            nc.vector.scalar_tensor_tensor(
                                o_sb[:, ct, :], t_sb[:], gbv_s[:, ct : ct + 1],
                                v1c[:, ct, :],
                                ALU.add, ALU.add,
                            )
                        # one 1MB output DMA per chunk (descriptor-efficient),
                        # alternating queues across chunks
                        [nc.sync, nc.scalar, nc.gpsimd][j % 3].dma_start(
                            outp[:, :, jw], o_sb[:]
                        )

                    ex_zero_rhs = vT_t[0][:, 0, :]
                    pend_epi = None
                    ngrp = mt // 4
                    exc = None
                    if "st" in drop:
                        exc = expp.tile([128, nwin], bf16, tag="exc", name="exc")
                        nc.vector.memset(exc[:], 0.01)
                    for j in range(nch):
                        jw = slice(j * nwin, (j + 1) * nwin)
                        # one PSUM tile (= one full bank) per output c-chunk:
                        # accumulation groups must not share a bank (start=True
                        # clears the whole bank's has_written bits)
                        accs = [
                            psA.tile([128, nwin], f32, tag=f"acc{ct}", name=f"acc{ct}")
                            for ct in range(CC)
                        ]
                        accl = psL.tile([128, nwin], f32, tag="accl")
                        v1cs = prefetch_v1(j)
                        # software pipeline over GROUPS of 4 m-tiles: issue
                        # S^T (row-packed pairs) + exp of group g before the
                        # P.V matmuls of group g-1, so ScalarE's exp overlaps
                        # TensorE's P.V.  The 4 denominator matmuls of a
                        # group are emitted back-to-back at PE column groups
                        # 0/32/64/96 (disjoint cells -> run concurrently).
                        prev_exs = None

                        def emit_st_pair(g, pi):
                            # one row-packed pair of S^T matmuls + their exps
                            out = []
                            if "st" in drop:
                                return [exc, exc]
                            for half in (0, 1):
                                m = 4 * g + 2 * pi + half
                                mj, ml = divmod(m, nwin // 128)
                                mw = slice(ml * 128, (ml + 1) * 128)
                                hp = slice(64 * half, 64 * half + 64)
                                st = psS.tile([128, nwin], f32, tag="st", name="st")
                                nc.tensor.matmul(
                                    st[:],
                                    kT_t[mj][hp, mw],
                                    qT_t[j][hp, :],
                                    start=True,
                                    stop=True,
                                    tile_position=(64 * half, 0),
                                )
                                ex = expp.tile([128, nwin], bf16, tag="ex", name="ex")
                                nc.scalar.activation(ex[:], st[:], AF.Exp)
                                out.append(ex)
                            return out

                        def emit_pv(g, lo, hi, exs):
                            # ct-outer: consecutive MMs share a PSUM bank
                            # (measured ~3% faster than alternating banks)
                            for ct in range(CC):
                                for i in range(lo, hi):
                                    m = 4 * g + i
                                    if "pv" in drop and m != 0:
                                        continue
                                    mj, ml = divmod(m, nwin // 128)
                                    nc.tensor.matmul(
                                        accs[ct][:],
                                        vT_t[mj][:, ml, ct * 128 : (ct + 1) * 128],
                                        exs[i][:],
                                        start=(m == 0),
                                        stop=(m == mt - 1 or "pv" in drop),
                                    )

                        for g in range(ngrp + 1):
                            exs = []
                            # interleave emission: S^T pair A, then half of
                            # the previous group's P.V, then pair B, then the
                            # rest + the denominator batch.  Keeps the 3rd/4th
                            # S^T (whose PSUM banks may still be feeding exp)
                            # from head-of-line-blocking 16 ready P.V matmuls
                            # in the PE FIFO.
                            if g < ngrp:
                                exs += emit_st_pair(g, 0)
                            if g == 1:
                                if pend_epi is not None:
                                    emit_epilogue(*pend_epi)
                                    pend_epi = None
                                # zero-weights matmul writes explicit zeros
                                # to the whole accl bank (start=True), so
                                # the column-tiled denominator chains can
                                # all accumulate with start=False (correct
                                # under both whole-bank and per-partition
                                # has_written semantics).  Emitted after the
                                # pipelined epilogue of the previous chunk
                                # so the shared psL slot is read first.
                                nc.tensor.matmul(
                                    accl[:], zeros_sq[:], ex_zero_rhs[:, :nwin],
                                    start=True, stop=("accl" in drop),
                                    skip_group_check=True,
                                )
                            if g > 0:
                                emit_pv(g - 1, 0, 2, prev_exs)
                            if g < ngrp:
                                exs += emit_st_pair(g, 1)
                            if g > 0:
                                emit_pv(g - 1, 2, 4, prev_exs)
                                if "accl" not in drop:
                                    for i in range(4):
                                        m = 4 * (g - 1) + i
                                        nc.tensor.matmul(
                                            accl[32 * i : 32 * i + 1, :],
                                            ones_col[:],
                                            prev_exs[i][:],
                                            start=False,
                                            stop=(g == ngrp),
                                            tile_position=(0, 32 * i),
                                            skip_group_check=True,
                                        )
                            prev_exs = exs
                        pend_epi = (j, accs, accl, v1cs)
                    emit_epilogue(*pend_epi)

        if repeat == 1:
            emit_rep(0)
        else:
            with tc.For_i(0, repeat, 1):
                emit_rep(0)

    nc.compile()
    return nc


def _get_nc(n=N, repeat=1):
    key = (n, repeat)
    if key not in _compiled:
        _compiled[key] = _build(n=n, repeat=repeat)
    return _compiled[key]


def _run(nc, view1, view2, Wq, bq, Wk, bk, Wv, bv, gamma, n=N, **spmd_kwargs):
    from concourse.bass_utils import run_bass_kernel_spmd

    b = view1.shape[0]
    f = np.ascontiguousarray
    gamma = np.asarray(gamma).astype(np.float32).reshape(-1)
    gbv = (gamma[0] * np.asarray(bv).astype(np.float32)).reshape(CC, 128).T
    import ml_dtypes

    com = {
        "wqT": f(Wq.T.astype(np.float16)),
        "wkT": f(Wk.T.astype(np.float16)),
        "wvT": f(Wv.T.astype(ml_dtypes.bfloat16)),
        "bqh": f(bq.astype(np.float32).reshape(D, 1)),
        "bkc": f(bk.astype(np.float32).reshape(D, 1)),
        "gbv": f(gbv),
        "gam": f(gamma.reshape(1, 1)),
    }
    in_maps = []
    for i in range(NCORES):
        bi = min(i, b - 1)  # replicate last sample if b < NCORES
        in_maps.append(
            {
                "v1": f(view1[bi].reshape(C, n).astype(np.float32)),
                "v2": f(view2[bi].reshape(C, n).astype(np.float32)),
                **com,
            }
        )
    res = run_bass_kernel_spmd(nc, in_maps, list(range(NCORES)), **spmd_kwargs)
    outs = [res.results[i]["out"] for i in range(b)]
    return np.stack(outs, axis=0)


def kernel(view1, view2, Wq, bq, Wk, bk, Wv, bv, gamma):
    view1 = np.asarray(view1)
    b, c, h, w = view1.shape
    n = h * w
    nc = _get_nc(n=n, repeat=1)
    out = _run(
        nc,
        np.asarray(view1),
        np.asarray(view2),
        np.asarray(Wq),
        np.asarray(bq),
        np.asarray(Wk),
        np.asarray(bk),
        np.asarray(Wv),
        np.asarray(bv),
        np.asarray(gamma),
        n=n,
    )
    return out.reshape(b, c, h, w).astype(np.float32)

